# revision 21
# baseline (speedup 1.0000x reference)
"""LoRADense (per-token adapter routing) Bass kernel for 8 Trainium2 NeuronCores.

Math (reference):
    base  = x @ kernel + bias                      # (N, F)
    a     = lora_a[adapter_ids]                    # (N, D, R) gather
    b     = lora_b[adapter_ids]                    # (N, R, F) gather
    lr    = einsum('nd,ndr->nr', x, a)             # (N, R)
    delta = einsum('nr,nrf->nf', lr, b)            # (N, F)
    out   = base + delta

Strategy:
  - Data parallel over tokens: 1024 tokens per core, all weights replicated.
  - Host sorts each core's tokens by adapter id (stable argsort). After
    sorting, each 128-token block's adapters fit inside a static window of
    W consecutive 128-row slabs of the concatenated LoRA matrices
    A_cat = lora_a.transpose(1,0,2).reshape(D, S*R)  (D, 1024)
    B_stk = lora_b.reshape(S*R, F)                   (1024, F)
    The window start slab sigma_b = clamp(b - W//2, 0, 8 - W) is the same for
    every core (SPMD-safe).  Containment is verified on the host; if it ever
    fails, W is widened (W=8 degenerates to the fully dense masked form,
    which is always correct).
  - Device per 128-token block b:
      stage A: lrT[sr_window, tok] = A_cat_slab^T-style matmuls (bf16),
               masked per (sr row, token) by is_equal(adapter_id, sr//16),
               result kept in SBUF as bf16.
      stage B: one PSUM accumulation per (block, f-half):
               8x f32r matmuls  (base: xsT^T @ kernel_slab)
             + Wx bf16 matmuls  (delta: lrm^T @ B_slab)
               then +bias on DVE and DMA to DRAM.
  - Host un-permutes the rows of the result.
"""

import numpy as np
import ml_dtypes

import concourse.bacc as bacc
import concourse.bass as bass
import concourse.mybir as mybir
import concourse.tile as tile
from concourse.bass_utils import run_bass_kernel_spmd

# Problem constants (hardcoded per harness contract).
N = 8192          # tokens
D = 1024          # input dim
F = 1024          # output features
R = 16            # lora rank
S = 64            # adapter slots
SR = S * R        # 1024
NCORES = 8
NTOK = N // NCORES            # 1024 tokens per core
P = 128                       # partitions
NT = NTOK // P                # 8 token blocks per core
KD = D // P                   # 8 contraction slabs over D
SRS = SR // P                 # 8 slabs over S*R
FH = 2                        # f halves of 512
FHW = F // FH                 # 512

BF16 = ml_dtypes.bfloat16

# Toggles (test.py pokes these).
TRACE = False
LAST_RESULTS = None
LAST_IN_MAPS = None
LAST_NC = None
LAST_W = None
LAST_SIGMAS = None
VARIANT = 3  # 1=base-only(f32r), 2=lora-only(bf16), 3=full
REPS = 1     # emit the whole compute this many times (benchmarking only)

_NC_CACHE = {}


def _sigmas_for(w):
    return tuple(min(max(b - w // 2, 0), SRS - w) for b in range(NT))


def _build_nc(w, sigmas):
    """Build the single-core Bass program (same program runs on all 8 cores)."""
    f32 = mybir.dt.float32
    f32r = mybir.dt.float32r
    bf16 = mybir.dt.bfloat16

    nc = bacc.Bacc("TRN2", target_bir_lowering=False, debug=False)

    # DRAM I/O. Layouts are pre-shuffled on the host so every DMA is a plain
    # contiguous [partition, free...] copy.
    xt = nc.dram_tensor("xt", [P, KD, NTOK], f32r, kind="ExternalInput")   # xsT: [p, d_o, tok]
    xtb = nc.dram_tensor("xtb", [P, KD, NTOK], bf16, kind="ExternalInput")  # xsT in bf16
    wk = nc.dram_tensor("wk", [P, KD, F], f32r, kind="ExternalInput")      # kernel: [p, d_o, f]
    ac = nc.dram_tensor("ac", [P, KD, SR], bf16, kind="ExternalInput")     # A_cat: [p, d_o, sr]
    bs = nc.dram_tensor("bs", [P, SRS, F], bf16, kind="ExternalInput")     # B_stk: [p, sr_o, f]
    idb = nc.dram_tensor("idb", [P, NTOK], f32, kind="ExternalInput")      # sorted ids bcast
    bib = nc.dram_tensor("bib", [P, F], f32, kind="ExternalInput")         # bias bcast
    adv = nc.dram_tensor("adv", [P, SRS], f32, kind="ExternalInput")       # (o*128+p)//16
    out_s = nc.dram_tensor("out_s", [NTOK, F], f32, kind="ExternalOutput")

    with tile.TileContext(nc) as tc:
        with (
            tc.tile_pool(name="const", bufs=1) as cpool,
            tc.tile_pool(name="work", bufs=4) as wpool,
            tc.tile_pool(name="lrps", bufs=4, space="PSUM") as lrps,
            tc.tile_pool(name="outps", bufs=4, space="PSUM") as outps,
        ):
            idb_sb = cpool.tile([P, NTOK], f32)
            nc.sync.dma_start(idb_sb[:], idb[:])
            bib_sb = cpool.tile([P, F], f32)
            nc.sync.dma_start(bib_sb[:], bib[:])
            adv_sb = cpool.tile([P, SRS], f32)
            nc.sync.dma_start(adv_sb[:], adv[:])

            # Per-slab DMAs so compute on slab k starts as soon as it lands.
            xt_sb = cpool.tile([P, KD, NTOK], f32r)
            xtb_sb = cpool.tile([P, KD, NTOK], bf16)
            wk_sb = cpool.tile([P, KD, F], f32r)
            ac_sb = cpool.tile([P, KD, SR], bf16)
            bs_sb = cpool.tile([P, SRS, F], bf16)
            for k in range(KD):
                nc.sync.dma_start(xtb_sb[:, k], xtb[:, k])
                nc.sync.dma_start(ac_sb[:, k], ac[:, k])
                nc.sync.dma_start(xt_sb[:, k], xt[:, k])
                nc.sync.dma_start(wk_sb[:, k], wk[:, k])
                nc.sync.dma_start(bs_sb[:, k], bs[:, k])

            # Masked low-rank activations, bf16: [sr_p, b*w + j, tok]
            lrm_sb = cpool.tile([P, NT * w, P], bf16)

            for b in [bb for _ in range(REPS) for bb in range(NT)]:
                sig = sigmas[b]
                tok = slice(b * P, (b + 1) * P)

                # ---- stage A: lrT window slabs + mask ----
                for j in range(w if VARIANT != 1 else 0):
                    o = sig + j
                    ps = lrps.tile([P, P], mybir.dt.float32, tag="lr")
                    if VARIANT == 5:
                        nc.vector.memset(ps[:], 0.0)
                    else:
                        for k in range(KD):
                            nc.tensor.matmul(
                                ps[:],
                                ac_sb[:, k, o * P:(o + 1) * P],
                                xtb_sb[:, k, tok],
                                start=(k == 0),
                                stop=(k == KD - 1),
                            )
                    if VARIANT == 4:
                        nc.vector.tensor_copy(out=lrm_sb[:, b * w + j], in_=ps[:])
                    else:
                        msk = wpool.tile([P, P], mybir.dt.float32, tag="msk")
                        # msk[p, t] = (ids[t] == (o*128+p)//16)
                        nc.vector.tensor_tensor(
                            msk[:],
                            idb_sb[:, tok],
                            adv_sb[:, o:o + 1].to_broadcast((P, P)),
                            mybir.AluOpType.is_equal,
                        )
                        nc.vector.tensor_tensor(
                            lrm_sb[:, b * w + j],
                            ps[:],
                            msk[:],
                            mybir.AluOpType.mult,
                        )

                # ---- stage B: fused base + delta accumulation ----
                for h in range(FH):
                    fs = slice(h * FHW, (h + 1) * FHW)
                    po = outps.tile([P, FHW], mybir.dt.float32, tag="out")
                    if VARIANT != 2:
                        for k in range(KD):
                            nc.tensor.matmul(
                                po[:],
                                xt_sb[:, k, tok],
                                wk_sb[:, k, fs],
                                start=(k == 0),
                                stop=(VARIANT == 1 and k == KD - 1),
                            )
                    if VARIANT != 1:
                        for j in range(w):
                            o = sig + j
                            nc.tensor.matmul(
                                po[:],
                                lrm_sb[:, b * w + j],
                                bs_sb[:, o, fs],
                                start=(VARIANT == 2 and j == 0),
                                stop=(j == w - 1),
                            )
                    ob = wpool.tile([P, FHW], mybir.dt.float32, tag="ob")
                    nc.vector.tensor_tensor(
                        ob[:], po[:], bib_sb[:, fs], mybir.AluOpType.add
                    )
                    nc.sync.dma_start(out_s[tok, fs], ob[:])

    nc.compile()
    return nc


def _get_nc(w, sigmas):
    key = (w, sigmas, VARIANT, REPS)
    if key not in _NC_CACHE:
        _NC_CACHE[key] = _build_nc(w, sigmas)
    return _NC_CACHE[key]


def kernel(x, adapter_ids, kernel, bias, lora_a, lora_b):
    global LAST_RESULTS
    x = np.ascontiguousarray(np.asarray(x, dtype=np.float32))
    adapter_ids = np.asarray(adapter_ids)
    kernel_w = np.ascontiguousarray(np.asarray(kernel, dtype=np.float32))
    bias = np.asarray(bias, dtype=np.float32)
    lora_a = np.asarray(lora_a, dtype=np.float32)
    lora_b = np.asarray(lora_b, dtype=np.float32)
    ids = adapter_ids.astype(np.int64)

    # Replicated weight layouts: [p, slab, free] with contiguous per-partition runs.
    a_cat = lora_a.transpose(1, 0, 2).reshape(D, SR)                  # (D, S*R)
    b_stk = lora_b.reshape(SR, F)                                     # (S*R, F)
    wk_l = np.ascontiguousarray(kernel_w.reshape(KD, P, F).transpose(1, 0, 2))
    ac_l = np.ascontiguousarray(
        a_cat.reshape(KD, P, SR).transpose(1, 0, 2).astype(BF16))
    bs_l = np.ascontiguousarray(
        b_stk.reshape(SRS, P, F).transpose(1, 0, 2).astype(BF16))
    bib_l = np.ascontiguousarray(np.broadcast_to(bias, (P, F)))
    adv_l = np.ascontiguousarray(
        ((np.arange(SRS)[None, :] * P + np.arange(P)[:, None]) // R)
        .astype(np.float32))

    # Per-core shards: sort tokens by adapter id.
    perms, in_maps = [], []
    ids_s_all = []
    for c in range(NCORES):
        lo = c * NTOK
        sh_ids = ids[lo:lo + NTOK]
        perm = np.argsort(sh_ids, kind="stable")
        perms.append(perm)
        ids_s = sh_ids[perm]
        ids_s_all.append(ids_s)
        xs = x[lo:lo + NTOK][perm]                                    # (NTOK, D)
        xt_l = np.ascontiguousarray(
            xs.T.reshape(KD, P, NTOK).transpose(1, 0, 2))             # (P, KD, NTOK)
        idb_l = np.ascontiguousarray(
            np.broadcast_to(ids_s.astype(np.float32), (P, NTOK)))
        in_maps.append({
            "xt": xt_l, "xtb": xt_l.astype(BF16), "wk": wk_l, "ac": ac_l,
            "bs": bs_l, "idb": idb_l, "bib": bib_l, "adv": adv_l,
        })

    # Pick the narrowest static window W whose containment holds on all cores.
    w_pick = None
    for w in (3, 4, 6, 8):
        sigmas = _sigmas_for(w)
        ok = True
        for ids_s in ids_s_all:
            for b in range(NT):
                blk = ids_s[b * P:(b + 1) * P]
                lo_a, hi_a = sigmas[b] * 8, (sigmas[b] + w) * 8
                if blk.min() < lo_a or blk.max() >= hi_a:
                    ok = False
                    break
            if not ok:
                break
        if ok:
            w_pick = w
            break
    assert w_pick is not None
    sigmas = _sigmas_for(w_pick)

    nc = _get_nc(w_pick, sigmas)
    res = run_bass_kernel_spmd(nc, in_maps, core_ids=list(range(NCORES)),
                               trace=TRACE)
    global LAST_IN_MAPS, LAST_NC, LAST_W, LAST_SIGMAS
    LAST_RESULTS = res
    LAST_IN_MAPS = in_maps
    LAST_NC = nc
    LAST_W = w_pick
    LAST_SIGMAS = sigmas

    out = np.empty((N, F), dtype=np.float32)
    for c in range(NCORES):
        seg = out[c * NTOK:(c + 1) * NTOK]
        seg[perms[c]] = res.results[c]["out_s"]
    return out


# revision 27
# speedup vs baseline: 12.2265x; 12.2265x over previous
"""LoRADense (per-token adapter routing) Bass kernel for 8 Trainium2 NeuronCores.

Math (reference):
    base  = x @ kernel + bias                      # (N, F)
    a     = lora_a[adapter_ids]                    # (N, D, R) gather
    b     = lora_b[adapter_ids]                    # (N, R, F) gather
    lr    = einsum('nd,ndr->nr', x, a)             # (N, R)
    delta = einsum('nr,nrf->nf', lr, b)            # (N, F)
    out   = base + delta

Strategy:
  - Data parallel over tokens: 1024 tokens per core, all weights replicated.
  - Host sorts each core's tokens by adapter id (stable argsort). After
    sorting, each 128-token block's adapters fit inside a static window of
    W consecutive 128-row slabs of the concatenated LoRA matrices
    A_cat = lora_a.transpose(1,0,2).reshape(D, S*R)  (D, 1024)
    B_stk = lora_b.reshape(S*R, F)                   (1024, F)
    The window start slab sigma_b = clamp(b - W//2, 0, 8 - W) is the same for
    every core (SPMD-safe).  Containment is verified on the host; if it ever
    fails, W is widened (W=8 degenerates to the fully dense masked form,
    which is always correct).
  - Device per 128-token block b:
      stage A: lrT[sr_window, tok] = A_cat_slab^T-style matmuls (bf16),
               masked per (sr row, token) by is_equal(adapter_id, sr//16),
               result kept in SBUF as bf16.
      stage B: one PSUM accumulation per (block, f-half):
               8x f32r matmuls  (base: xsT^T @ kernel_slab)
             + Wx bf16 matmuls  (delta: lrm^T @ B_slab)
               then +bias on DVE and DMA to DRAM.
  - Host un-permutes the rows of the result.
"""

import numpy as np
import ml_dtypes

import concourse.bacc as bacc
import concourse.bass as bass
import concourse.mybir as mybir
import concourse.tile as tile
from concourse.bass_utils import run_bass_kernel_spmd

# Problem constants (hardcoded per harness contract).
N = 8192          # tokens
D = 1024          # input dim
F = 1024          # output features
R = 16            # lora rank
S = 64            # adapter slots
SR = S * R        # 1024
NCORES = 8
NTOK = N // NCORES            # 1024 tokens per core
P = 128                       # partitions
NT = NTOK // P                # 8 token blocks per core
KD = D // P                   # 8 contraction slabs over D
SRS = SR // P                 # 8 slabs over S*R
FH = 2                        # f halves of 512
FHW = F // FH                 # 512

BF16 = ml_dtypes.bfloat16

# Toggles (test.py pokes these).
TRACE = False
LAST_RESULTS = None
LAST_IN_MAPS = None
LAST_NC = None
LAST_W = None
LAST_SIGMAS = None
VARIANT = 3  # 1=base-only(f32r), 2=lora-only(bf16), 3=full
REPS = 1     # emit the whole compute this many times (benchmarking only)

_NC_CACHE = {}


def _sigmas_for(w):
    return tuple(min(max(b - w // 2, 0), SRS - w) for b in range(NT))


def _build_nc(w, sigmas):
    """Build the single-core Bass program (same program runs on all 8 cores)."""
    f32 = mybir.dt.float32
    f32r = mybir.dt.float32r
    bf16 = mybir.dt.bfloat16

    nc = bacc.Bacc("TRN2", target_bir_lowering=False, debug=False)

    # DRAM I/O. Layouts are pre-shuffled on the host so every DMA is a plain
    # contiguous [partition, free...] copy.
    xt = nc.dram_tensor("xt", [P, KD, NTOK], f32r, kind="ExternalInput")   # xsT: [p, d_o, tok]
    xtb = nc.dram_tensor("xtb", [P, KD, NTOK], bf16, kind="ExternalInput")  # xsT in bf16
    wk = nc.dram_tensor("wk", [P, KD, F], f32r, kind="ExternalInput")      # kernel: [p, d_o, f]
    ac = nc.dram_tensor("ac", [P, KD, SR], bf16, kind="ExternalInput")     # A_cat: [p, d_o, sr]
    bs = nc.dram_tensor("bs", [P, SRS, F], bf16, kind="ExternalInput")     # B_stk: [p, sr_o, f]
    msk = nc.dram_tensor("msk", [P, NT * w, P], f32, kind="ExternalInput")  # host masks
    bib = nc.dram_tensor("bib", [P, F], f32, kind="ExternalInput")         # bias bcast
    out_s = nc.dram_tensor("out_s", [NTOK, F], f32, kind="ExternalOutput")

    with tile.TileContext(nc) as tc:
        with (
            tc.tile_pool(name="const", bufs=1) as cpool,
            tc.tile_pool(name="work", bufs=4) as wpool,
            tc.tile_pool(name="lrps", bufs=4, space="PSUM") as lrps,
            tc.tile_pool(name="outps", bufs=4, space="PSUM") as outps,
        ):
            msk_sb = cpool.tile([P, NT * w, P], f32)
            nc.sync.dma_start(msk_sb[:], msk[:])
            bib_sb = cpool.tile([P, F], f32)
            nc.sync.dma_start(bib_sb[:], bib[:])

            # Per-slab DMAs so compute on slab k starts as soon as it lands.
            xt_sb = cpool.tile([P, KD, NTOK], f32r)
            xtb_sb = cpool.tile([P, KD, NTOK], bf16)
            wk_sb = cpool.tile([P, KD, F], f32r)
            ac_sb = cpool.tile([P, KD, SR], bf16)
            bs_sb = cpool.tile([P, SRS, F], bf16)
            for k in range(KD):
                nc.sync.dma_start(xtb_sb[:, k], xtb[:, k])
                nc.sync.dma_start(ac_sb[:, k], ac[:, k])
                nc.sync.dma_start(xt_sb[:, k], xt[:, k])
                nc.sync.dma_start(wk_sb[:, k], wk[:, k])
                nc.sync.dma_start(bs_sb[:, k], bs[:, k])

            # Masked low-rank activations, bf16: [sr_p, b*w + j, tok]
            lrm_sb = cpool.tile([P, NT * w, P], bf16)

            for b in [bb for _ in range(REPS) for bb in range(NT)]:
                sig = sigmas[b]
                tok = slice(b * P, (b + 1) * P)

                # ---- stage A: lrT window slabs + mask ----
                for j in range(w if VARIANT != 1 else 0):
                    o = sig + j
                    ps = lrps.tile([P, P], mybir.dt.float32, tag="lr")
                    if VARIANT == 5:
                        nc.vector.memset(ps[:], 0.0)
                    else:
                        for k in range(KD):
                            nc.tensor.matmul(
                                ps[:],
                                ac_sb[:, k, o * P:(o + 1) * P],
                                xtb_sb[:, k, tok],
                                start=(k == 0),
                                stop=(k == KD - 1),
                            )
                    if VARIANT == 4:
                        nc.vector.tensor_copy(out=lrm_sb[:, b * w + j], in_=ps[:])
                    else:
                        # msk[p, b*w+j, t] = (ids[t] == (o*128+p)//16), host-built
                        nc.vector.tensor_tensor(
                            lrm_sb[:, b * w + j],
                            ps[:],
                            msk_sb[:, b * w + j],
                            mybir.AluOpType.mult,
                        )

                # ---- stage B: fused base + delta accumulation ----
                for h in range(FH):
                    fs = slice(h * FHW, (h + 1) * FHW)
                    po = outps.tile([P, FHW], mybir.dt.float32, tag="out")
                    if VARIANT != 2:
                        for k in range(KD):
                            nc.tensor.matmul(
                                po[:],
                                xt_sb[:, k, tok],
                                wk_sb[:, k, fs],
                                start=(k == 0),
                                stop=(VARIANT == 1 and k == KD - 1),
                            )
                    if VARIANT != 1:
                        for j in range(w):
                            o = sig + j
                            nc.tensor.matmul(
                                po[:],
                                lrm_sb[:, b * w + j],
                                bs_sb[:, o, fs],
                                start=(VARIANT == 2 and j == 0),
                                stop=(j == w - 1),
                            )
                    ob = wpool.tile([P, FHW], mybir.dt.float32, tag="ob")
                    nc.any.tensor_tensor(
                        ob[:], po[:], bib_sb[:, fs], mybir.AluOpType.add
                    )
                    nc.sync.dma_start(out_s[tok, fs], ob[:])

    nc.compile()
    return nc


def _get_nc(w, sigmas):
    key = (w, sigmas, VARIANT, REPS)
    if key not in _NC_CACHE:
        _NC_CACHE[key] = _build_nc(w, sigmas)
    return _NC_CACHE[key]


def kernel(x, adapter_ids, kernel, bias, lora_a, lora_b):
    global LAST_RESULTS
    x = np.ascontiguousarray(np.asarray(x, dtype=np.float32))
    adapter_ids = np.asarray(adapter_ids)
    kernel_w = np.ascontiguousarray(np.asarray(kernel, dtype=np.float32))
    bias = np.asarray(bias, dtype=np.float32)
    lora_a = np.asarray(lora_a, dtype=np.float32)
    lora_b = np.asarray(lora_b, dtype=np.float32)
    ids = adapter_ids.astype(np.int64)

    # Replicated weight layouts: [p, slab, free] with contiguous per-partition runs.
    a_cat = lora_a.transpose(1, 0, 2).reshape(D, SR)                  # (D, S*R)
    b_stk = lora_b.reshape(SR, F)                                     # (S*R, F)
    wk_l = np.ascontiguousarray(kernel_w.reshape(KD, P, F).transpose(1, 0, 2))
    ac_l = np.ascontiguousarray(
        a_cat.reshape(KD, P, SR).transpose(1, 0, 2).astype(BF16))
    bs_l = np.ascontiguousarray(
        b_stk.reshape(SRS, P, F).transpose(1, 0, 2).astype(BF16))
    bib_l = np.ascontiguousarray(np.broadcast_to(bias, (P, F)))

    # Per-core shards: sort tokens by adapter id.
    perms, ids_s_all = [], []
    for c in range(NCORES):
        lo = c * NTOK
        sh_ids = ids[lo:lo + NTOK]
        perm = np.argsort(sh_ids, kind="stable")
        perms.append(perm)
        ids_s_all.append(sh_ids[perm])

    # Pick the narrowest static window W whose containment holds on all cores.
    w_pick = None
    for w in (3, 4, 6, 8):
        sigmas = _sigmas_for(w)
        ok = True
        for ids_s in ids_s_all:
            for b in range(NT):
                blk = ids_s[b * P:(b + 1) * P]
                lo_a, hi_a = sigmas[b] * 8, (sigmas[b] + w) * 8
                if blk.min() < lo_a or blk.max() >= hi_a:
                    ok = False
                    break
            if not ok:
                break
        if ok:
            w_pick = w
            break
    assert w_pick is not None
    sigmas = _sigmas_for(w_pick)

    # Per-(slab-row, window-slab) adapter index: adiv[p, o] = (o*128+p)//16
    adiv = (np.arange(SRS)[None, :] * P + np.arange(P)[:, None]) // R  # (P, SRS)
    in_maps = []
    for c in range(NCORES):
        lo = c * NTOK
        ids_s = ids_s_all[c]
        xs = x[lo:lo + NTOK][perms[c]]                                # (NTOK, D)
        xt_l = np.ascontiguousarray(
            xs.T.reshape(KD, P, NTOK).transpose(1, 0, 2))             # (P, KD, NTOK)
        # msk[p, b*w+j, t] = (ids_s[b*128+t] == (sigma_b+j)*8 + p//16)
        slabs = np.array([sigmas[b] + j for b in range(NT)
                          for j in range(w_pick)])                    # (NT*w,)
        ids_blk = ids_s.reshape(NT, P)                                # (NT, P)
        ids_rep = np.repeat(ids_blk, w_pick, axis=0)                  # (NT*w, P)
        msk_l = np.ascontiguousarray(
            (adiv[:, slabs][:, :, None] == ids_rep[None, :, :])
            .astype(np.float32))                                      # (P, NT*w, P)
        in_maps.append({
            "xt": xt_l, "xtb": xt_l.astype(BF16), "wk": wk_l, "ac": ac_l,
            "bs": bs_l, "msk": msk_l, "bib": bib_l,
        })

    nc = _get_nc(w_pick, sigmas)
    res = run_bass_kernel_spmd(nc, in_maps, core_ids=list(range(NCORES)),
                               trace=TRACE)
    global LAST_IN_MAPS, LAST_NC, LAST_W, LAST_SIGMAS
    LAST_RESULTS = res
    LAST_IN_MAPS = in_maps
    LAST_NC = nc
    LAST_W = w_pick
    LAST_SIGMAS = sigmas

    out = np.empty((N, F), dtype=np.float32)
    for c in range(NCORES):
        seg = out[c * NTOK:(c + 1) * NTOK]
        seg[perms[c]] = res.results[c]["out_s"]
    return out
